# revision 20
# baseline (speedup 1.0000x reference)
"""Bass/Tile Trainium2 kernel for nn_Attention (B=4, T=4096, C=256), 8 cores.

Sharding: core = (batch b, query-half h). Each core computes attention
output for its 2048 query rows against the batch's valid keys.

Key ideas:
  - Masked keys contribute exactly zero to the reference softmax (the
    -inf mask), and attention is permutation-invariant over keys, so the
    host packs only the VALID key columns (padded to a 128 multiple,
    TK). With the ~50% random mask this halves the score/out matmuls
    and the exp work. The kernel is compiled lazily for the TK the
    actual mask needs and cached; any mask value is handled correctly.
  - Wk is folded into the query side: scores = k.q = (Wk x).q =
    x.(Wk^T q), so q'' = Wk^T-contract(q^T) costs TQ columns instead of
    a k projection over all keys — packed raw x^T serves as the key
    matrix for the score matmuls.
  - The pad/valid 0/1 column is applied on device: the V psum->SBUF
    copy is a per-partition tensor_scalar_mul by the mask column, and
    the appended ones column of V is the mask itself, so pad keys drop
    out of both softmax sums (their x columns are zero anyway). The
    torch quirk (+1.0 bias on valid keys) cancels in softmax.
  - V gets the mask column appended: out[q, 256] accumulates the
    softmax denominator for free. Final: out[:, :256] * (1/out[:, 256]),
    emitted as bf16 (host upcasts to f32).
  - Softmax needs no max-subtraction (exp of O(10) scores cannot
    overflow fp32) and no partition reductions.
  - Main loop is software-pipelined per key block with depth-2 score
    prefetch: PE emits scores for jb+2 before the out-matmuls of jb, so
    ACT's exp (~600 ns/tile) has ~1.7 us of PE cover.

All matmuls bf16 with fp32 PSUM accumulation (fp8 fails the 2e-2
error gate: e4m3 p needs bias -6 which flushes the low tail, e5m2 has
too little mantissa, and fp8 q/k adds ~5% worst-row error).
"""

import numpy as np
import ml_dtypes

import concourse.bacc as bacc
import concourse.mybir as mybir
import concourse.tile as tile
from concourse.bass_utils import run_bass_kernel_spmd

B, T, C = 4, 4096, 256
NCORES = 8
HALVES = NCORES // B          # 2 query-halves per batch
TQ = T // HALVES              # 2048 query rows per core
PB = 128                      # partition block
NCCH = C // PB                # 2 contraction chunks of 128
SBW = 512                     # query superblock width
NSB = TQ // SBW               # 4 superblocks per core
NQB = SBW // PB               # 4 query 128-blocks per superblock
VW = C + 1                    # v tile width incl. mask/ones column
SCALE = float(C) ** -0.5
BF16 = mybir.dt.bfloat16
F32 = mybir.dt.float32
# Stage score psum -> SBUF bf16 on DVE so ACT's exp reads SBUF: halves the
# psum-read dwell (DVE 2x 16-bit-out rate) to cut ACT<->PE psum contention.
STAGE_DVE = True


def _emit(tc, out, xq, xk, wq, wk, wv, mb, tk, mode="full"):
    nc = tc.nc
    njb = tk // PB
    import contextlib

    with contextlib.ExitStack() as ctx:
        persist = ctx.enter_context(tc.tile_pool(name="persist", bufs=1))
        # Persistent SBUF tensors; c-chunks laid side by side on the free dim.
        xq_sb = persist.tile([PB, NCCH * TQ], BF16)   # x^T  (query slice)
        xk_sb = persist.tile([PB, NCCH * tk], BF16)   # x^T  (packed valid keys)
        wq_sb = persist.tile([PB, NCCH * C], BF16)
        wk_sb = persist.tile([PB, NCCH * C], BF16)    # raw Wk, d on partitions
        wv_sb = persist.tile([PB, NCCH * C], BF16)
        qt_sb = persist.tile([PB, NCCH * TQ], BF16)   # q^T
        q2_sb = persist.tile([PB, NCCH * TQ], BF16)   # q''^T = Wk^T q^T
        va_sb = persist.tile([PB, njb * VW], BF16)    # masked v + mask col
        mb_sb = persist.tile([PB, njb], F32)          # 0/1 valid mask [j, jb]

        # Few, large, descriptor-friendly DMAs spread across the three
        # DMA-capable queues (sync/scalar HWDGE, gpsimd SWDGE). Weights and
        # xq land first so the q projection starts while xk streams.
        w2 = lambda w: w.rearrange("(n p) c -> p n c", p=PB)
        s3 = lambda t, n: t.rearrange("p (n c) -> p n c", n=n)
        nc.scalar.dma_start(s3(wq_sb[:], NCCH), w2(wq))
        nc.sync.dma_start(s3(wk_sb[:], NCCH), w2(wk))
        nc.gpsimd.dma_start(s3(wv_sb[:], NCCH), w2(wv))
        nc.gpsimd.dma_start(mb_sb[:], mb)
        nc.sync.dma_start(xq_sb[:, 0:TQ], xq[0:PB, :])
        nc.scalar.dma_start(xq_sb[:, TQ:2 * TQ], xq[PB:2 * PB, :])
        nc.sync.dma_start(xk_sb[:, 0:tk], xk[0:PB, :])
        nc.scalar.dma_start(xk_sb[:, tk:2 * tk], xk[PB:2 * PB, :])

        # mask/ones column: va[:, jb*VW + C] = mask01[:, jb]
        va_ones = va_sb[:].rearrange("p (j e) -> p j e", e=VW)[:, :, C:C + 1]
        nc.vector.tensor_copy(va_ones, mb_sb[:].rearrange("p (j e) -> p j e", e=1))

        # ---- projections ----
        with tc.tile_pool(name="proj_psum", bufs=2, space="PSUM") as pp:
            # q^T[d, t]: lhsT = W^T chunk [c, d], rhs = x^T [c, tq-slice]
            # q''^T[c, t]: lhsT = Wk chunk [d, c], rhs = q^T [d, tq-slice]
            for w_sb, x_src, dst in (
                (wq_sb, xq_sb, qt_sb),
                (wk_sb, qt_sb, q2_sb),
            ):
                for s in range(TQ // 512):
                    for dc in range(NCCH):
                        ps = pp.tile([PB, 512], F32, tag="proj", name="proj_ps")
                        for cc in range(NCCH):
                            nc.tensor.matmul(
                                ps,
                                lhsT=w_sb[:, cc * C + dc * PB: cc * C + (dc + 1) * PB],
                                rhs=x_src[:, cc * TQ + s * 512: cc * TQ + (s + 1) * 512],
                                start=(cc == 0),
                                stop=(cc == NCCH - 1),
                            )
                        nc.vector.tensor_copy(
                            dst[:, dc * TQ + s * 512: dc * TQ + (s + 1) * 512], ps)
            # v[t, d]: lhsT = x^T chunk [c, t-block], rhs = W^T chunk [c, d].
            # The psum->SBUF copy applies the valid-key mask per partition.
            for jb in range(njb):
                ps = pp.tile([PB, C], F32, tag="projv", name="projv_ps")
                for cc in range(NCCH):
                    nc.tensor.matmul(
                        ps,
                        lhsT=xk_sb[:, cc * tk + jb * PB: cc * tk + (jb + 1) * PB],
                        rhs=wv_sb[:, cc * C:(cc + 1) * C],
                        start=(cc == 0),
                        stop=(cc == NCCH - 1),
                    )
                nc.vector.tensor_scalar_mul(
                    va_sb[:, jb * VW: jb * VW + C], ps, mb_sb[:, jb:jb + 1])

        # ---- attention main loop ----
        # Key blocks are processed in PAIRS: the two score matmul groups of a
        # pair write the two banks of one [128, 1024] psum tile, and a single
        # wide exp covers both — halving ACT instruction count/overhead.
        npair = (njb + 1) // 2
        scp = ctx.enter_context(tc.tile_pool(name="sc_psum", bufs=2, space="PSUM"))
        op = ctx.enter_context(tc.tile_pool(name="o_psum", bufs=1, space="PSUM"))
        ppool = ctx.enter_context(tc.tile_pool(name="p_pool", bufs=3))
        spool = ctx.enter_context(tc.tile_pool(name="s_pool", bufs=3))
        fin = ctx.enter_context(tc.tile_pool(name="fin", bufs=3))

        if mode == "noscores":
            p_static = persist.tile([PB, SBW], BF16, name="p_static")
            nc.vector.memset(p_static[:], 1.0)

        for sb in range(NSB):
            op_tiles = [op.tile([PB, VW], F32, tag=f"o{qb}", name=f"opsum{qb}")
                        for qb in range(NQB)]
            p_tiles = {}

            def emit_scores(jp, sb=sb, p_tiles=p_tiles):
                jbs = [jb for jb in (2 * jp, 2 * jp + 1) if jb < njb]
                w = len(jbs) * SBW
                ps = scp.tile([PB, 2 * SBW], F32, tag="sc", name="sc_ps")
                for i, jb in enumerate(jbs):
                    for cc in range(NCCH):
                        nc.tensor.matmul(
                            ps[:, i * SBW:(i + 1) * SBW],
                            lhsT=xk_sb[:, cc * tk + jb * PB: cc * tk + (jb + 1) * PB],
                            rhs=q2_sb[:, cc * TQ + sb * SBW: cc * TQ + (sb + 1) * SBW],
                            start=(cc == 0),
                            stop=(cc == NCCH - 1),
                        )
                pt = ppool.tile([PB, 2 * SBW], BF16, tag="p", name="p_t")
                if STAGE_DVE:
                    st = spool.tile([PB, 2 * SBW], BF16, tag="ss", name="ss_t")
                    nc.vector.tensor_copy(st[:, 0:w], ps[:, 0:w])
                    nc.scalar.activation(
                        pt[:, 0:w], st[:, 0:w],
                        mybir.ActivationFunctionType.Exp, scale=SCALE)
                else:
                    nc.scalar.activation(
                        pt[:, 0:w], ps[:, 0:w],
                        mybir.ActivationFunctionType.Exp, scale=SCALE)
                p_tiles[jp] = pt

            def emit_out(jp, op_tiles=op_tiles, p_tiles=p_tiles):
                # qb-major so consecutive matmuls accumulate into the SAME
                # psum bank (fewer accumulation-target switches on PE).
                pt = p_tiles.pop(jp) if mode != "noscores" else None
                for qb in range(NQB):
                    for jb in (2 * jp, 2 * jp + 1):
                        if jb >= njb:
                            continue
                        i = jb - 2 * jp
                        lhsT = (pt[:, i * SBW + qb * PB: i * SBW + (qb + 1) * PB]
                                if pt is not None else
                                p_static[:, qb * PB:(qb + 1) * PB])
                        nc.tensor.matmul(
                            op_tiles[qb],
                            lhsT=lhsT,
                            rhs=va_sb[:, jb * VW:(jb + 1) * VW],
                            start=(jb == 0),
                            stop=(jb == njb - 1),
                        )

            if mode == "noout":
                for jp in range(npair):
                    emit_scores(jp)
                    p_tiles.pop(jp)
            elif mode == "noscores":
                for jp in range(npair):
                    emit_out(jp)
            else:
                # depth-1 pair prefetch = 2 key blocks of lookahead: scores
                # for pair jp+1 are emitted before the out-matmuls of jp, so
                # ACT's wide exp (~1.1us) has ~1.7us of PE cover.
                emit_scores(0)
                for jp in range(npair):
                    if jp + 1 < npair:
                        emit_scores(jp + 1)
                    emit_out(jp)
            if mode == "noout":
                os_t = fin.tile([PB, C], BF16, tag="os", name="os_t")
                nc.vector.tensor_copy(os_t, qt_sb[:, sb * C:(sb + 1) * C])
                nc.sync.dma_start(out[sb * PB:(sb + 1) * PB, :], os_t)
                continue
            os_t = fin.tile([PB, NQB * C], BF16, tag="os", name="os_t")
            for qb in range(NQB):
                rec = fin.tile([PB, 1], F32, tag="rec", name="rec_t")
                nc.vector.reciprocal(rec, op_tiles[qb][:, C:C + 1])
                nc.vector.tensor_scalar_mul(
                    os_t[:, qb * C:(qb + 1) * C], op_tiles[qb][:, 0:C], rec)
            dma_eng = nc.sync if sb % 2 == 0 else nc.scalar
            dma_eng.dma_start(
                out[sb * SBW:(sb + 1) * SBW, :].rearrange("(q p) c -> p q c", p=PB),
                os_t[:].rearrange("p (q c) -> p q c", q=NQB))


def build_nc(reps=1, loop_n=0, mode="full", tk=2176, unroll=1):
    nc = bacc.Bacc("TRN2", target_bir_lowering=False, debug=False)
    xq = nc.dram_tensor("xq", [C, TQ], BF16, kind="ExternalInput").ap()
    xk = nc.dram_tensor("xk", [C, tk], BF16, kind="ExternalInput").ap()
    wq = nc.dram_tensor("wq", [C, C], BF16, kind="ExternalInput").ap()
    wk = nc.dram_tensor("wk", [C, C], BF16, kind="ExternalInput").ap()
    wv = nc.dram_tensor("wv", [C, C], BF16, kind="ExternalInput").ap()
    mb = nc.dram_tensor("mb", [PB, tk // PB], F32, kind="ExternalInput").ap()
    out = nc.dram_tensor("out", [TQ, C], BF16, kind="ExternalOutput").ap()
    with tile.TileContext(nc) as tc:
        if loop_n:
            assert loop_n % unroll == 0
            with tc.For_i(0, loop_n // unroll, 1,
                          hint_engines=(mybir.EngineType.PE,)):
                for _ in range(unroll):
                    _emit(tc, out, xq, xk, wq, wk, wv, mb, tk, mode=mode)
        else:
            for _ in range(reps):
                _emit(tc, out, xq, xk, wq, wk, wv, mb, tk, mode=mode)
    nc.compile()
    return nc


_CACHE = {}


def _get_nc(tk):
    if tk not in _CACHE:
        _CACHE[tk] = build_nc(tk=tk)
    return _CACHE[tk]


def make_in_maps(x, mask):
    """Per-core input maps. Packs valid keys; returns (maps, tk)."""
    bf = ml_dtypes.bfloat16
    x = np.asarray(x, dtype=np.float32)
    xt_all = np.ascontiguousarray(x.transpose(0, 2, 1)).astype(bf)  # [B, C, T]
    m01 = np.asarray(mask) != 0                                     # [B, T]
    nvalid = m01.sum(axis=1)
    tk = max(PB, int(-(-int(nvalid.max()) // PB) * PB))
    xks, mbs = [], []
    for b in range(B):
        nv = int(nvalid[b])
        xk = np.zeros((C, tk), dtype=bf)
        xk[:, :nv] = xt_all[b][:, m01[b]]
        mp = np.zeros(tk, dtype=np.float32)
        mp[:nv] = 1.0
        xks.append(xk)
        mbs.append(np.ascontiguousarray(mp.reshape(tk // PB, PB).T))
    maps = []
    for core in range(NCORES):
        b, h = divmod(core, HALVES)
        maps.append({
            "xq": np.ascontiguousarray(xt_all[b][:, h * TQ:(h + 1) * TQ]),
            "xk": xks[b],
            "mb": mbs[b],
        })
    return maps, tk


def kernel(x, mask, Wk, Wq, Wv):
    bf = ml_dtypes.bfloat16
    wqt = np.ascontiguousarray(np.asarray(Wq, dtype=np.float32).T).astype(bf)
    wk_raw = np.ascontiguousarray(np.asarray(Wk, dtype=np.float32)).astype(bf)
    wvt = np.ascontiguousarray(np.asarray(Wv, dtype=np.float32).T).astype(bf)
    in_maps, tk = make_in_maps(x, mask)
    for m in in_maps:
        m.update({"wq": wqt, "wk": wk_raw, "wv": wvt})
    res = run_bass_kernel_spmd(_get_nc(tk), in_maps, list(range(NCORES)))
    out = np.empty((B, T, C), np.float32)
    for core in range(NCORES):
        b, h = divmod(core, HALVES)
        out[b, h * TQ:(h + 1) * TQ, :] = res.results[core]["out"]
    return out


# revision 21
# speedup vs baseline: 1.0549x; 1.0549x over previous
"""Bass/Tile Trainium2 kernel for nn_Attention (B=4, T=4096, C=256), 8 cores.

Sharding: core = (batch b, query-half h). Each core computes attention
output for its 2048 query rows against the batch's valid keys.

Key ideas:
  - Masked keys contribute exactly zero to the reference softmax (the
    -inf mask), and attention is permutation-invariant over keys, so the
    host packs only the VALID key columns (padded to a 128 multiple,
    TK). With the ~50% random mask this halves the score/out matmuls
    and the exp work. The kernel is compiled lazily for the TK the
    actual mask needs and cached; any mask value is handled correctly.
  - Wk is folded into the query side: scores = k.q = (Wk x).q =
    x.(Wk^T q), so q'' = Wk^T-contract(q^T) costs TQ columns instead of
    a k projection over all keys — packed raw x^T serves as the key
    matrix for the score matmuls.
  - The pad/valid 0/1 column is applied on device: the V psum->SBUF
    copy is a per-partition tensor_scalar_mul by the mask column, and
    the appended ones column of V is the mask itself, so pad keys drop
    out of both softmax sums (their x columns are zero anyway). The
    torch quirk (+1.0 bias on valid keys) cancels in softmax.
  - V gets the mask column appended: out[q, 256] accumulates the
    softmax denominator for free. Final: out[:, :256] * (1/out[:, 256]),
    emitted as bf16 (host upcasts to f32).
  - Softmax needs no max-subtraction (exp of O(10) scores cannot
    overflow fp32) and no partition reductions.
  - Main loop is software-pipelined per key block with depth-2 score
    prefetch: PE emits scores for jb+2 before the out-matmuls of jb, so
    ACT's exp (~600 ns/tile) has ~1.7 us of PE cover.

All matmuls bf16 with fp32 PSUM accumulation (fp8 fails the 2e-2
error gate: e4m3 p needs bias -6 which flushes the low tail, e5m2 has
too little mantissa, and fp8 q/k adds ~5% worst-row error).
"""

import numpy as np
import ml_dtypes

import concourse.bacc as bacc
import concourse.mybir as mybir
import concourse.tile as tile
from concourse.bass_utils import run_bass_kernel_spmd

B, T, C = 4, 4096, 256
NCORES = 8
HALVES = NCORES // B          # 2 query-halves per batch
TQ = T // HALVES              # 2048 query rows per core
PB = 128                      # partition block
NCCH = C // PB                # 2 contraction chunks of 128
SBW = 512                     # query superblock width
NSB = TQ // SBW               # 4 superblocks per core
NQB = SBW // PB               # 4 query 128-blocks per superblock
VW = C + 1                    # v tile width incl. mask/ones column
SCALE = float(C) ** -0.5
BF16 = mybir.dt.bfloat16
F32 = mybir.dt.float32
# Stage score psum -> SBUF bf16 on DVE so ACT's exp reads SBUF. Tested on
# HW: ~9us WORSE than exp reading psum directly (extra pass + bf16 scores
# also cost accuracy: 0.0094 vs 0.0051 rel err). Keep False.
STAGE_DVE = False


def _emit(tc, out, xq, xk, wq, wk, wv, mb, tk, mode="full"):
    nc = tc.nc
    njb = tk // PB
    import contextlib

    with contextlib.ExitStack() as ctx:
        persist = ctx.enter_context(tc.tile_pool(name="persist", bufs=1))
        # Persistent SBUF tensors; c-chunks laid side by side on the free dim.
        xq_sb = persist.tile([PB, NCCH * TQ], BF16)   # x^T  (query slice)
        xk_sb = persist.tile([PB, NCCH * tk], BF16)   # x^T  (packed valid keys)
        wq_sb = persist.tile([PB, NCCH * C], BF16)
        wk_sb = persist.tile([PB, NCCH * C], BF16)    # raw Wk, d on partitions
        wv_sb = persist.tile([PB, NCCH * C], BF16)
        qt_sb = persist.tile([PB, NCCH * TQ], BF16)   # q^T
        q2_sb = persist.tile([PB, NCCH * TQ], BF16)   # q''^T = Wk^T q^T
        va_sb = persist.tile([PB, njb * VW], BF16)    # masked v + mask col
        mb_sb = persist.tile([PB, njb], F32)          # 0/1 valid mask [j, jb]

        # Few, large, descriptor-friendly DMAs spread across the three
        # DMA-capable queues (sync/scalar HWDGE, gpsimd SWDGE). Weights and
        # xq land first so the q projection starts while xk streams.
        w2 = lambda w: w.rearrange("(n p) c -> p n c", p=PB)
        s3 = lambda t, n: t.rearrange("p (n c) -> p n c", n=n)
        nc.scalar.dma_start(s3(wq_sb[:], NCCH), w2(wq))
        nc.sync.dma_start(s3(wk_sb[:], NCCH), w2(wk))
        nc.gpsimd.dma_start(s3(wv_sb[:], NCCH), w2(wv))
        nc.gpsimd.dma_start(mb_sb[:], mb)
        nc.sync.dma_start(xq_sb[:, 0:TQ], xq[0:PB, :])
        nc.scalar.dma_start(xq_sb[:, TQ:2 * TQ], xq[PB:2 * PB, :])
        nc.sync.dma_start(xk_sb[:, 0:tk], xk[0:PB, :])
        nc.scalar.dma_start(xk_sb[:, tk:2 * tk], xk[PB:2 * PB, :])

        # mask/ones column: va[:, jb*VW + C] = mask01[:, jb]
        va_ones = va_sb[:].rearrange("p (j e) -> p j e", e=VW)[:, :, C:C + 1]
        nc.vector.tensor_copy(va_ones, mb_sb[:].rearrange("p (j e) -> p j e", e=1))

        # ---- projections ----
        with tc.tile_pool(name="proj_psum", bufs=2, space="PSUM") as pp:
            # q^T[d, t]: lhsT = W^T chunk [c, d], rhs = x^T [c, tq-slice]
            # q''^T[c, t]: lhsT = Wk chunk [d, c], rhs = q^T [d, tq-slice]
            for w_sb, x_src, dst in (
                (wq_sb, xq_sb, qt_sb),
                (wk_sb, qt_sb, q2_sb),
            ):
                for s in range(TQ // 512):
                    for dc in range(NCCH):
                        ps = pp.tile([PB, 512], F32, tag="proj", name="proj_ps")
                        for cc in range(NCCH):
                            nc.tensor.matmul(
                                ps,
                                lhsT=w_sb[:, cc * C + dc * PB: cc * C + (dc + 1) * PB],
                                rhs=x_src[:, cc * TQ + s * 512: cc * TQ + (s + 1) * 512],
                                start=(cc == 0),
                                stop=(cc == NCCH - 1),
                            )
                        nc.vector.tensor_copy(
                            dst[:, dc * TQ + s * 512: dc * TQ + (s + 1) * 512], ps)
            # v[t, d]: lhsT = x^T chunk [c, t-block], rhs = W^T chunk [c, d].
            # The psum->SBUF copy applies the valid-key mask per partition.
            for jb in range(njb):
                ps = pp.tile([PB, C], F32, tag="projv", name="projv_ps")
                for cc in range(NCCH):
                    nc.tensor.matmul(
                        ps,
                        lhsT=xk_sb[:, cc * tk + jb * PB: cc * tk + (jb + 1) * PB],
                        rhs=wv_sb[:, cc * C:(cc + 1) * C],
                        start=(cc == 0),
                        stop=(cc == NCCH - 1),
                    )
                nc.vector.tensor_scalar_mul(
                    va_sb[:, jb * VW: jb * VW + C], ps, mb_sb[:, jb:jb + 1])

        # ---- attention main loop ----
        # Key blocks are processed in PAIRS: the two score matmul groups of a
        # pair write the two banks of one [128, 1024] psum tile, and a single
        # wide exp covers both — halving ACT instruction count/overhead.
        npair = (njb + 1) // 2
        scp = ctx.enter_context(tc.tile_pool(name="sc_psum", bufs=2, space="PSUM"))
        op = ctx.enter_context(tc.tile_pool(name="o_psum", bufs=1, space="PSUM"))
        ppool = ctx.enter_context(tc.tile_pool(name="p_pool", bufs=3))
        spool = ctx.enter_context(tc.tile_pool(name="s_pool", bufs=3))
        fin = ctx.enter_context(tc.tile_pool(name="fin", bufs=3))

        if mode == "noscores":
            p_static = persist.tile([PB, SBW], BF16, name="p_static")
            nc.vector.memset(p_static[:], 1.0)

        for sb in range(NSB):
            op_tiles = [op.tile([PB, VW], F32, tag=f"o{qb}", name=f"opsum{qb}")
                        for qb in range(NQB)]
            p_tiles = {}

            def emit_scores(jp, sb=sb, p_tiles=p_tiles):
                jbs = [jb for jb in (2 * jp, 2 * jp + 1) if jb < njb]
                w = len(jbs) * SBW
                ps = scp.tile([PB, 2 * SBW], F32, tag="sc", name="sc_ps")
                for i, jb in enumerate(jbs):
                    for cc in range(NCCH):
                        nc.tensor.matmul(
                            ps[:, i * SBW:(i + 1) * SBW],
                            lhsT=xk_sb[:, cc * tk + jb * PB: cc * tk + (jb + 1) * PB],
                            rhs=q2_sb[:, cc * TQ + sb * SBW: cc * TQ + (sb + 1) * SBW],
                            start=(cc == 0),
                            stop=(cc == NCCH - 1),
                        )
                pt = ppool.tile([PB, 2 * SBW], BF16, tag="p", name="p_t")
                if STAGE_DVE:
                    st = spool.tile([PB, 2 * SBW], BF16, tag="ss", name="ss_t")
                    nc.vector.tensor_copy(st[:, 0:w], ps[:, 0:w])
                    nc.scalar.activation(
                        pt[:, 0:w], st[:, 0:w],
                        mybir.ActivationFunctionType.Exp, scale=SCALE)
                else:
                    nc.scalar.activation(
                        pt[:, 0:w], ps[:, 0:w],
                        mybir.ActivationFunctionType.Exp, scale=SCALE)
                p_tiles[jp] = pt

            def emit_out(jp, op_tiles=op_tiles, p_tiles=p_tiles):
                # qb-major so consecutive matmuls accumulate into the SAME
                # psum bank (fewer accumulation-target switches on PE).
                pt = p_tiles.pop(jp) if mode != "noscores" else None
                for qb in range(NQB):
                    for jb in (2 * jp, 2 * jp + 1):
                        if jb >= njb:
                            continue
                        i = jb - 2 * jp
                        lhsT = (pt[:, i * SBW + qb * PB: i * SBW + (qb + 1) * PB]
                                if pt is not None else
                                p_static[:, qb * PB:(qb + 1) * PB])
                        nc.tensor.matmul(
                            op_tiles[qb],
                            lhsT=lhsT,
                            rhs=va_sb[:, jb * VW:(jb + 1) * VW],
                            start=(jb == 0),
                            stop=(jb == njb - 1),
                        )

            if mode == "noout":
                for jp in range(npair):
                    emit_scores(jp)
                    p_tiles.pop(jp)
            elif mode == "noscores":
                for jp in range(npair):
                    emit_out(jp)
            else:
                # depth-1 pair prefetch = 2 key blocks of lookahead: scores
                # for pair jp+1 are emitted before the out-matmuls of jp, so
                # ACT's wide exp (~1.1us) has ~1.7us of PE cover.
                emit_scores(0)
                for jp in range(npair):
                    if jp + 1 < npair:
                        emit_scores(jp + 1)
                    emit_out(jp)
            if mode == "noout":
                os_t = fin.tile([PB, C], BF16, tag="os", name="os_t")
                nc.vector.tensor_copy(os_t, qt_sb[:, sb * C:(sb + 1) * C])
                nc.sync.dma_start(out[sb * PB:(sb + 1) * PB, :], os_t)
                continue
            os_t = fin.tile([PB, NQB * C], BF16, tag="os", name="os_t")
            for qb in range(NQB):
                rec = fin.tile([PB, 1], F32, tag="rec", name="rec_t")
                nc.vector.reciprocal(rec, op_tiles[qb][:, C:C + 1])
                nc.vector.tensor_scalar_mul(
                    os_t[:, qb * C:(qb + 1) * C], op_tiles[qb][:, 0:C], rec)
            dma_eng = nc.sync if sb % 2 == 0 else nc.scalar
            dma_eng.dma_start(
                out[sb * SBW:(sb + 1) * SBW, :].rearrange("(q p) c -> p q c", p=PB),
                os_t[:].rearrange("p (q c) -> p q c", q=NQB))


def build_nc(reps=1, loop_n=0, mode="full", tk=2176, unroll=1):
    nc = bacc.Bacc("TRN2", target_bir_lowering=False, debug=False)
    xq = nc.dram_tensor("xq", [C, TQ], BF16, kind="ExternalInput").ap()
    xk = nc.dram_tensor("xk", [C, tk], BF16, kind="ExternalInput").ap()
    wq = nc.dram_tensor("wq", [C, C], BF16, kind="ExternalInput").ap()
    wk = nc.dram_tensor("wk", [C, C], BF16, kind="ExternalInput").ap()
    wv = nc.dram_tensor("wv", [C, C], BF16, kind="ExternalInput").ap()
    mb = nc.dram_tensor("mb", [PB, tk // PB], F32, kind="ExternalInput").ap()
    out = nc.dram_tensor("out", [TQ, C], BF16, kind="ExternalOutput").ap()
    with tile.TileContext(nc) as tc:
        if loop_n:
            assert loop_n % unroll == 0
            with tc.For_i(0, loop_n // unroll, 1,
                          hint_engines=(mybir.EngineType.PE,)):
                for _ in range(unroll):
                    _emit(tc, out, xq, xk, wq, wk, wv, mb, tk, mode=mode)
        else:
            for _ in range(reps):
                _emit(tc, out, xq, xk, wq, wk, wv, mb, tk, mode=mode)
    nc.compile()
    return nc


_CACHE = {}


def _get_nc(tk):
    if tk not in _CACHE:
        _CACHE[tk] = build_nc(tk=tk)
    return _CACHE[tk]


def make_in_maps(x, mask):
    """Per-core input maps. Packs valid keys; returns (maps, tk)."""
    bf = ml_dtypes.bfloat16
    x = np.asarray(x, dtype=np.float32)
    xt_all = np.ascontiguousarray(x.transpose(0, 2, 1)).astype(bf)  # [B, C, T]
    m01 = np.asarray(mask) != 0                                     # [B, T]
    nvalid = m01.sum(axis=1)
    tk = max(PB, int(-(-int(nvalid.max()) // PB) * PB))
    xks, mbs = [], []
    for b in range(B):
        nv = int(nvalid[b])
        xk = np.zeros((C, tk), dtype=bf)
        xk[:, :nv] = xt_all[b][:, m01[b]]
        mp = np.zeros(tk, dtype=np.float32)
        mp[:nv] = 1.0
        xks.append(xk)
        mbs.append(np.ascontiguousarray(mp.reshape(tk // PB, PB).T))
    maps = []
    for core in range(NCORES):
        b, h = divmod(core, HALVES)
        maps.append({
            "xq": np.ascontiguousarray(xt_all[b][:, h * TQ:(h + 1) * TQ]),
            "xk": xks[b],
            "mb": mbs[b],
        })
    return maps, tk


def kernel(x, mask, Wk, Wq, Wv):
    bf = ml_dtypes.bfloat16
    wqt = np.ascontiguousarray(np.asarray(Wq, dtype=np.float32).T).astype(bf)
    wk_raw = np.ascontiguousarray(np.asarray(Wk, dtype=np.float32)).astype(bf)
    wvt = np.ascontiguousarray(np.asarray(Wv, dtype=np.float32).T).astype(bf)
    in_maps, tk = make_in_maps(x, mask)
    for m in in_maps:
        m.update({"wq": wqt, "wk": wk_raw, "wv": wvt})
    res = run_bass_kernel_spmd(_get_nc(tk), in_maps, list(range(NCORES)))
    out = np.empty((B, T, C), np.float32)
    for core in range(NCORES):
        b, h = divmod(core, HALVES)
        out[b, h * TQ:(h + 1) * TQ, :] = res.results[core]["out"]
    return out


# revision 24
# speedup vs baseline: 1.0778x; 1.0217x over previous
"""Bass/Tile Trainium2 kernel for nn_Attention (B=4, T=4096, C=256), 8 cores.

Sharding: core = (batch b, query-half h). Each core computes attention
output for its 2048 query rows against the batch's valid keys.

Key ideas:
  - Masked keys contribute exactly zero to the reference softmax (the
    -inf mask), and attention is permutation-invariant over keys, so the
    host packs only the VALID key columns (padded to a 128 multiple,
    TK). With the ~50% random mask this halves the score/out matmuls
    and the exp work. The kernel is compiled lazily for the TK the
    actual mask needs and cached; any mask value is handled correctly.
  - Wk is folded into the query side: scores = k.q = (Wk x).q =
    x.(Wk^T q), so q'' = Wk^T-contract(q^T) costs TQ columns instead of
    a k projection over all keys — packed raw x^T serves as the key
    matrix for the score matmuls.
  - The pad/valid 0/1 column is applied on device: the V psum->SBUF
    copy is a per-partition tensor_scalar_mul by the mask column, and
    the appended ones column of V is the mask itself, so pad keys drop
    out of both softmax sums (their x columns are zero anyway). The
    torch quirk (+1.0 bias on valid keys) cancels in softmax.
  - V gets the mask column appended: out[q, 256] accumulates the
    softmax denominator for free. Final: out[:, :256] * (1/out[:, 256]),
    emitted as bf16 (host upcasts to f32).
  - Softmax needs no max-subtraction (exp of O(10) scores cannot
    overflow fp32) and no partition reductions.
  - Main loop is software-pipelined per key block with depth-2 score
    prefetch: PE emits scores for jb+2 before the out-matmuls of jb, so
    ACT's exp (~600 ns/tile) has ~1.7 us of PE cover.

All matmuls bf16 with fp32 PSUM accumulation (fp8 fails the 2e-2
error gate: e4m3 p needs bias -6 which flushes the low tail, e5m2 has
too little mantissa, and fp8 q/k adds ~5% worst-row error).
"""

import numpy as np
import ml_dtypes

import concourse.bacc as bacc
import concourse.mybir as mybir
import concourse.tile as tile
from concourse.bass_utils import run_bass_kernel_spmd

B, T, C = 4, 4096, 256
NCORES = 8
HALVES = NCORES // B          # 2 query-halves per batch
TQ = T // HALVES              # 2048 query rows per core
PB = 128                      # partition block
NCCH = C // PB                # 2 contraction chunks of 128
SBW = 512                     # query superblock width
NSB = TQ // SBW               # 4 superblocks per core
NQB = SBW // PB               # 4 query 128-blocks per superblock
VW = C + 1                    # v tile width incl. mask/ones column
SCALE = float(C) ** -0.5
BF16 = mybir.dt.bfloat16
F32 = mybir.dt.float32
# Stage score psum -> SBUF bf16 on DVE so ACT's exp reads SBUF. Tested on
# HW: ~9us WORSE than exp reading psum directly (extra pass + bf16 scores
# also cost accuracy: 0.0094 vs 0.0051 rel err). Keep False.
STAGE_DVE = False


def _emit(tc, out, xq, xk, wq, wk, wv, mb, tk, mode="full"):
    nc = tc.nc
    njb = tk // PB
    import contextlib

    with contextlib.ExitStack() as ctx:
        persist = ctx.enter_context(tc.tile_pool(name="persist", bufs=1))
        # Persistent SBUF tensors; c-chunks laid side by side on the free dim.
        xq_sb = persist.tile([PB, NCCH * TQ], BF16)   # x^T  (query slice)
        xk_sb = persist.tile([PB, NCCH * tk], BF16)   # x^T  (packed valid keys)
        wq_sb = persist.tile([PB, NCCH * C], BF16)
        wk_sb = persist.tile([PB, NCCH * C], BF16)    # raw Wk, d on partitions
        wv_sb = persist.tile([PB, NCCH * C], BF16)
        qt_sb = persist.tile([PB, NCCH * TQ], BF16)   # q^T
        q2_sb = persist.tile([PB, NCCH * TQ], BF16)   # q''^T = Wk^T q^T
        va_sb = persist.tile([PB, njb * VW], BF16)    # masked v + mask col
        mb_sb = persist.tile([PB, njb], F32)          # 0/1 valid mask [j, jb]

        # Few, large, descriptor-friendly DMAs spread across the three
        # DMA-capable queues (sync/scalar HWDGE, gpsimd SWDGE). Weights and
        # xq land first so the q projection starts while xk streams.
        w2 = lambda w: w.rearrange("(n p) c -> p n c", p=PB)
        s3 = lambda t, n: t.rearrange("p (n c) -> p n c", n=n)
        nc.scalar.dma_start(s3(wq_sb[:], NCCH), w2(wq))
        nc.sync.dma_start(s3(wk_sb[:], NCCH), w2(wk))
        nc.gpsimd.dma_start(s3(wv_sb[:], NCCH), w2(wv))
        nc.gpsimd.dma_start(mb_sb[:], mb)
        nc.sync.dma_start(xq_sb[:, 0:TQ], xq[0:PB, :])
        nc.scalar.dma_start(xq_sb[:, TQ:2 * TQ], xq[PB:2 * PB, :])
        nc.sync.dma_start(xk_sb[:, 0:tk], xk[0:PB, :])
        nc.scalar.dma_start(xk_sb[:, tk:2 * tk], xk[PB:2 * PB, :])

        # mask/ones column: va[:, jb*VW + C] = mask01[:, jb]
        va_ones = va_sb[:].rearrange("p (j e) -> p j e", e=VW)[:, :, C:C + 1]
        nc.vector.tensor_copy(va_ones, mb_sb[:].rearrange("p (j e) -> p j e", e=1))

        # ---- projections ----
        # psum->SBUF copies alternate DVE/ACT: the copy chain (~500ns each)
        # is slower than PE's 427ns per stripe, and ACT is idle here.
        with tc.tile_pool(name="proj_psum", bufs=3, space="PSUM") as pp:
            # q^T[d, t]: lhsT = W^T chunk [c, d], rhs = x^T [c, tq-slice]
            # q''^T[c, t]: lhsT = Wk chunk [d, c], rhs = q^T [d, tq-slice]
            for w_sb, x_src, dst in (
                (wq_sb, xq_sb, qt_sb),
                (wk_sb, qt_sb, q2_sb),
            ):
                for s in range(TQ // 512):
                    for dc in range(NCCH):
                        ps = pp.tile([PB, 512], F32, tag="proj", name="proj_ps")
                        for cc in range(NCCH):
                            nc.tensor.matmul(
                                ps,
                                lhsT=w_sb[:, cc * C + dc * PB: cc * C + (dc + 1) * PB],
                                rhs=x_src[:, cc * TQ + s * 512: cc * TQ + (s + 1) * 512],
                                start=(cc == 0),
                                stop=(cc == NCCH - 1),
                            )
                        dst_ap = dst[:, dc * TQ + s * 512: dc * TQ + (s + 1) * 512]
                        if (s * NCCH + dc) % 2 == 0:
                            nc.vector.tensor_copy(dst_ap, ps)
                        else:
                            nc.scalar.copy(dst_ap, ps)
            # v[t, d]: lhsT = x^T chunk [c, t-block], rhs = W^T chunk [c, d].
            # The psum->SBUF copy applies the valid-key mask per partition.
            for jb in range(njb):
                ps = pp.tile([PB, C], F32, tag="projv", name="projv_ps")
                for cc in range(NCCH):
                    nc.tensor.matmul(
                        ps,
                        lhsT=xk_sb[:, cc * tk + jb * PB: cc * tk + (jb + 1) * PB],
                        rhs=wv_sb[:, cc * C:(cc + 1) * C],
                        start=(cc == 0),
                        stop=(cc == NCCH - 1),
                    )
                va_ap = va_sb[:, jb * VW: jb * VW + C]
                if jb % 2 == 0:
                    nc.vector.tensor_scalar_mul(va_ap, ps, mb_sb[:, jb:jb + 1])
                else:
                    nc.scalar.mul(va_ap, ps, mb_sb[:, jb:jb + 1])

        # ---- attention main loop ----
        # Key blocks are processed in PAIRS: the two score matmul groups of a
        # pair write the two banks of one [128, 1024] psum tile, and a single
        # wide exp covers both — halving ACT instruction count/overhead.
        # The FIRST group of each superblock is a single block: its shorter
        # exp is ready in time for the first out-matmuls, absorbing the
        # sb-start latency (a full pair's exp lacks ~0.5us of PE cover there).
        groups = [[0]]
        i = 1
        while i < njb:
            groups.append([i] if i + 1 >= njb else [i, i + 1])
            i += 2
        ngr = len(groups)
        scp = ctx.enter_context(tc.tile_pool(name="sc_psum", bufs=2, space="PSUM"))
        op = ctx.enter_context(tc.tile_pool(name="o_psum", bufs=1, space="PSUM"))
        ppool = ctx.enter_context(tc.tile_pool(name="p_pool", bufs=3))
        spool = ctx.enter_context(tc.tile_pool(name="s_pool", bufs=3))
        fin = ctx.enter_context(tc.tile_pool(name="fin", bufs=3))

        if mode == "noscores":
            p_static = persist.tile([PB, SBW], BF16, name="p_static")
            nc.vector.memset(p_static[:], 1.0)

        for sb in range(NSB):
            op_tiles = [op.tile([PB, VW], F32, tag=f"o{qb}", name=f"opsum{qb}")
                        for qb in range(NQB)]
            p_tiles = {}

            def emit_scores(gi, sb=sb, p_tiles=p_tiles):
                jbs = groups[gi]
                w = len(jbs) * SBW
                ps = scp.tile([PB, 2 * SBW], F32, tag="sc", name="sc_ps")
                for i, jb in enumerate(jbs):
                    for cc in range(NCCH):
                        nc.tensor.matmul(
                            ps[:, i * SBW:(i + 1) * SBW],
                            lhsT=xk_sb[:, cc * tk + jb * PB: cc * tk + (jb + 1) * PB],
                            rhs=q2_sb[:, cc * TQ + sb * SBW: cc * TQ + (sb + 1) * SBW],
                            start=(cc == 0),
                            stop=(cc == NCCH - 1),
                        )
                pt = ppool.tile([PB, 2 * SBW], BF16, tag="p", name="p_t")
                if STAGE_DVE:
                    st = spool.tile([PB, 2 * SBW], BF16, tag="ss", name="ss_t")
                    nc.vector.tensor_copy(st[:, 0:w], ps[:, 0:w])
                    nc.scalar.activation(
                        pt[:, 0:w], st[:, 0:w],
                        mybir.ActivationFunctionType.Exp, scale=SCALE)
                else:
                    nc.scalar.activation(
                        pt[:, 0:w], ps[:, 0:w],
                        mybir.ActivationFunctionType.Exp, scale=SCALE)
                p_tiles[gi] = pt

            def emit_out(gi, op_tiles=op_tiles, p_tiles=p_tiles):
                # qb-major so consecutive matmuls accumulate into the SAME
                # psum bank (fewer accumulation-target switches on PE).
                pt = p_tiles.pop(gi) if mode != "noscores" else None
                for qb in range(NQB):
                    for i, jb in enumerate(groups[gi]):
                        lhsT = (pt[:, i * SBW + qb * PB: i * SBW + (qb + 1) * PB]
                                if pt is not None else
                                p_static[:, qb * PB:(qb + 1) * PB])
                        nc.tensor.matmul(
                            op_tiles[qb],
                            lhsT=lhsT,
                            rhs=va_sb[:, jb * VW:(jb + 1) * VW],
                            start=(jb == 0),
                            stop=(jb == njb - 1),
                        )

            if mode == "noout":
                for gi in range(ngr):
                    emit_scores(gi)
                    p_tiles.pop(gi)
            elif mode == "noscores":
                for gi in range(ngr):
                    emit_out(gi)
            else:
                # depth-1 group prefetch: scores for group gi+1 are emitted
                # before the out-matmuls of gi, so ACT's wide exp (~1.1us)
                # has ~1.7us of PE cover.
                emit_scores(0)
                for gi in range(ngr):
                    if gi + 1 < ngr:
                        emit_scores(gi + 1)
                    emit_out(gi)
            if mode == "noout":
                os_t = fin.tile([PB, C], BF16, tag="os", name="os_t")
                nc.vector.tensor_copy(os_t, qt_sb[:, sb * C:(sb + 1) * C])
                nc.sync.dma_start(out[sb * PB:(sb + 1) * PB, :], os_t)
                continue
            os_t = fin.tile([PB, NQB * C], BF16, tag="os", name="os_t")
            for qb in range(NQB):
                rec = fin.tile([PB, 1], F32, tag="rec", name="rec_t")
                nc.vector.reciprocal(rec, op_tiles[qb][:, C:C + 1])
                nc.vector.tensor_scalar_mul(
                    os_t[:, qb * C:(qb + 1) * C], op_tiles[qb][:, 0:C], rec)
            dma_eng = nc.sync if sb % 2 == 0 else nc.scalar
            dma_eng.dma_start(
                out[sb * SBW:(sb + 1) * SBW, :].rearrange("(q p) c -> p q c", p=PB),
                os_t[:].rearrange("p (q c) -> p q c", q=NQB))


def build_nc(reps=1, loop_n=0, mode="full", tk=2176, unroll=1):
    nc = bacc.Bacc("TRN2", target_bir_lowering=False, debug=False)
    xq = nc.dram_tensor("xq", [C, TQ], BF16, kind="ExternalInput").ap()
    xk = nc.dram_tensor("xk", [C, tk], BF16, kind="ExternalInput").ap()
    wq = nc.dram_tensor("wq", [C, C], BF16, kind="ExternalInput").ap()
    wk = nc.dram_tensor("wk", [C, C], BF16, kind="ExternalInput").ap()
    wv = nc.dram_tensor("wv", [C, C], BF16, kind="ExternalInput").ap()
    mb = nc.dram_tensor("mb", [PB, tk // PB], F32, kind="ExternalInput").ap()
    out = nc.dram_tensor("out", [TQ, C], BF16, kind="ExternalOutput").ap()
    with tile.TileContext(nc) as tc:
        if loop_n:
            assert loop_n % unroll == 0
            with tc.For_i(0, loop_n // unroll, 1,
                          hint_engines=(mybir.EngineType.PE,)):
                for _ in range(unroll):
                    _emit(tc, out, xq, xk, wq, wk, wv, mb, tk, mode=mode)
        else:
            for _ in range(reps):
                _emit(tc, out, xq, xk, wq, wk, wv, mb, tk, mode=mode)
    nc.compile()
    return nc


_CACHE = {}


def _get_nc(tk):
    if tk not in _CACHE:
        _CACHE[tk] = build_nc(tk=tk)
    return _CACHE[tk]


def make_in_maps(x, mask):
    """Per-core input maps. Packs valid keys; returns (maps, tk)."""
    bf = ml_dtypes.bfloat16
    x = np.asarray(x, dtype=np.float32)
    xt_all = np.ascontiguousarray(x.transpose(0, 2, 1)).astype(bf)  # [B, C, T]
    m01 = np.asarray(mask) != 0                                     # [B, T]
    nvalid = m01.sum(axis=1)
    tk = max(PB, int(-(-int(nvalid.max()) // PB) * PB))
    xks, mbs = [], []
    for b in range(B):
        nv = int(nvalid[b])
        xk = np.zeros((C, tk), dtype=bf)
        xk[:, :nv] = xt_all[b][:, m01[b]]
        mp = np.zeros(tk, dtype=np.float32)
        mp[:nv] = 1.0
        xks.append(xk)
        mbs.append(np.ascontiguousarray(mp.reshape(tk // PB, PB).T))
    maps = []
    for core in range(NCORES):
        b, h = divmod(core, HALVES)
        maps.append({
            "xq": np.ascontiguousarray(xt_all[b][:, h * TQ:(h + 1) * TQ]),
            "xk": xks[b],
            "mb": mbs[b],
        })
    return maps, tk


def kernel(x, mask, Wk, Wq, Wv):
    bf = ml_dtypes.bfloat16
    wqt = np.ascontiguousarray(np.asarray(Wq, dtype=np.float32).T).astype(bf)
    wk_raw = np.ascontiguousarray(np.asarray(Wk, dtype=np.float32)).astype(bf)
    wvt = np.ascontiguousarray(np.asarray(Wv, dtype=np.float32).T).astype(bf)
    in_maps, tk = make_in_maps(x, mask)
    for m in in_maps:
        m.update({"wq": wqt, "wk": wk_raw, "wv": wvt})
    res = run_bass_kernel_spmd(_get_nc(tk), in_maps, list(range(NCORES)))
    out = np.empty((B, T, C), np.float32)
    for core in range(NCORES):
        b, h = divmod(core, HALVES)
        out[b, h * TQ:(h + 1) * TQ, :] = res.results[core]["out"]
    return out
